# revision 8
# baseline (speedup 1.0000x reference)
"""Trainium2 Bass kernel for nn_ComplexCrossAttention.

Sharding: 8 cores = 2 batches x 4 head-groups (4 heads each).
Each core computes, for its (b, head-group):
  - complex Q/K/V projections (column-sharded by head) in transposed layout
  - attention scoresT = (qr.kr + qi.ki)*scale with s on partitions
  - softmax (no max-subtraction; scores are provably small) via exp + column-sum
  - av in transposed layout -> OT [d2, l]
  - partial output projection (row-sharded by head)
Host sums the 4 partial y per batch and adds the bias.

All matmuls are N=512 full-rate. Activations enter as bf16 (DMA-xbar
transpose requires 2-byte dtype); later stages use float32r (TF32-like,
full rate at N>=256).
"""

import sys

import numpy as np

try:
    import concourse.bacc as bacc
except ImportError:  # pragma: no cover - fallback for bare environments
    sys.path.insert(0, "/opt/trn_rl_repo")
    import concourse.bacc as bacc

import concourse.mybir as mybir
import concourse.tile as tile
from concourse.bass_utils import run_bass_kernel_spmd

F32 = mybir.dt.float32
BF16 = mybir.dt.bfloat16
F32R = mybir.dt.float32r

# ---- problem constants (hardcoded per contract) ----
B, L, S, C = 2, 2048, 2048, 1024
H, D = 16, 64
SCALE = float(1.0 / np.sqrt(np.float32(D)))
HPC = 4          # heads per core
D2 = 2 * D       # stacked (real|imag) head dim = 128
NCK = C // 128   # contraction chunks = 8
NLB = L // 512   # l-blocks = 4
NST = S // 128   # s-tiles = 16
NLT = L // 128   # l-tiles = 16
NEB = 2          # e-blocks of 512 in C

# ---- dtype configuration ----
QS_DT = F32R     # Qs/Ks (scores operands)
EXP_DT = BF16    # expT / Vs / ones (av + denom operands)
VS_DT = EXP_DT
OT_DT = F32R     # OT / wo (o-proj operands)

_CACHE = {}


def _build_program():
    nc = bacc.Bacc("TRN2", target_bir_lowering=False, debug=False, num_devices=8)

    # per-core external inputs
    x_r = nc.dram_tensor("x_r", [L, C], F32, kind="ExternalInput")
    x_i = nc.dram_tensor("x_i", [L, C], F32, kind="ExternalInput")
    c_r = nc.dram_tensor("c_r", [S, C], F32, kind="ExternalInput")
    c_i = nc.dram_tensor("c_i", [S, C], F32, kind="ExternalInput")
    # stacked complex projection weights (host-prepared, bf16)
    # wq/wk: [C, HPC, 2, 128]  (c, head, pm, m) ; lhsT tiles
    wq = nc.dram_tensor("wq", [C, HPC, 2, D2], BF16, kind="ExternalInput")
    wk = nc.dram_tensor("wk", [C, HPC, 2, D2], BF16, kind="ExternalInput")
    # wv: [C, 2, HPC*128]  (c, pm, all-head d2) ; rhs tiles
    wv = nc.dram_tensor("wv", [C, 2, HPC * D2], BF16, kind="ExternalInput")
    # wo: [HPC, 128, 2, NEB, 512]  (head, d2row, ri, eblock, e) ; rhs tiles, f32
    wo = nc.dram_tensor("wo", [HPC, D2, 2, NEB, 512], F32, kind="ExternalInput")

    y_r = nc.dram_tensor("y_r", [L, C], F32, kind="ExternalOutput")
    y_i = nc.dram_tensor("y_i", [L, C], F32, kind="ExternalOutput")

    with tile.TileContext(nc) as tc:
        _emit(nc, tc, x_r, x_i, c_r, c_i, wq, wk, wv, wo, y_r, y_i)

    nc.compile()
    return nc


def _emit(nc, tc, x_r, x_i, c_r, c_i, wq, wk, wv, wo, y_r, y_i):
    import os
    from contextlib import ExitStack

    max_phase = int(os.environ.get("KMAX_PHASE", "9"))

    ctx = ExitStack()
    with ctx:
        singles = ctx.enter_context(tc.tile_pool(name="singles", bufs=1))
        attn_sb = ctx.enter_context(tc.tile_pool(name="attn_sb", bufs=1))
        dram = ctx.enter_context(tc.tile_pool(name="dram", bufs=1, space="DRAM"))

        # bf16 bounce buffers for the xbar transpose
        xbf = [dram.tile([L, C], BF16, tag=f"xbf{t}", name=f"xbf{t}") for t in range(2)]
        cbf = [dram.tile([S, C], BF16, tag=f"cbf{t}", name=f"cbf{t}") for t in range(2)]

        # ---------- P0: cast fp32 -> bf16 and bounce to DRAM ----------
        with tc.tile_pool(name="cast", bufs=4) as cast:
            for asrc, adst in ((x_r, xbf[0]), (x_i, xbf[1]), (c_r, cbf[0]), (c_i, cbf[1])):
                for lt in range(NLT):
                    t32 = cast.tile([128, C], F32, tag="cast32")
                    nc.sync.dma_start(out=t32, in_=asrc[lt * 128:(lt + 1) * 128, :])
                    t16 = cast.tile([128, C], BF16, tag="cast16")
                    nc.gpsimd.tensor_copy(out=t16, in_=t32)
                    nc.sync.dma_start(out=adst[lt * 128:(lt + 1) * 128, :], in_=t16)

        # persistent attention operands
        qs = attn_sb.tile([128, HPC, L], QS_DT)            # [d2, h, l]
        ks = attn_sb.tile([128, HPC, S], QS_DT)            # [d2, h, s]
        vs = attn_sb.tile([128, NST, HPC * D2], VS_DT)     # [s-part, st, d2all]

        # ---------- P1+P2: transpose-in x, Q projection ----------
        if max_phase < 2:
            return
        with (
            tc.tile_pool(name="xt", bufs=1) as xt_pool,
            tc.tile_pool(name="wqk", bufs=1) as wqk_pool,
            tc.tile_pool(name="ps_proj", bufs=1, space="PSUM") as ps_proj,
        ):
            xt = [xt_pool.tile([128, NCK, L], BF16, tag=f"xt{t}", name=f"xt{t}") for t in range(2)]
            for t in range(2):
                for ck in range(NCK):
                    nc.sync.dma_start(
                        out=xt[t][:, ck, :],
                        in_=xbf[t][:, ck * 128:(ck + 1) * 128],
                        transpose=True,
                    )
            wq_sb = wqk_pool.tile([128, NCK, HPC, 2, D2], BF16, tag="wq")
            nc.sync.dma_start(
                out=wq_sb, in_=wq.rearrange("(ck p) h pm m -> p ck h pm m", p=128)
            )
            for h in range(HPC):
                pq = [ps_proj.tile([128, 512], F32, tag=f"pq{lb}", name=f"pq{lb}") for lb in range(NLB)]
                n = 2 * NCK
                i = 0
                for ck in range(NCK):
                    for pm in range(2):
                        for lb in range(NLB):
                            nc.tensor.matmul(
                                pq[lb],
                                wq_sb[:, ck, h, pm, :],
                                xt[pm][:, ck, lb * 512:(lb + 1) * 512],
                                start=(i == 0),
                                stop=(i == n - 1),
                            )
                        i += 1
                for lb in range(NLB):
                    nc.vector.tensor_copy(out=qs[:, h, lb * 512:(lb + 1) * 512], in_=pq[lb])

        # ---------- P3: transpose-in ctx, K and V projections ----------
        if max_phase < 3:
            return
        with (
            tc.tile_pool(name="ct", bufs=1) as ct_pool,
            tc.tile_pool(name="wkv", bufs=1) as wkv_pool,
            tc.tile_pool(name="ps_proj2", bufs=1, space="PSUM") as ps_proj,
            tc.tile_pool(name="ps_v", bufs=2, space="PSUM") as ps_v,
        ):
            ct = [ct_pool.tile([128, NCK, S], BF16, tag=f"ct{t}", name=f"ct{t}") for t in range(2)]
            for t in range(2):
                for ck in range(NCK):
                    nc.sync.dma_start(
                        out=ct[t][:, ck, :],
                        in_=cbf[t][:, ck * 128:(ck + 1) * 128],
                        transpose=True,
                    )
            wk_sb = wkv_pool.tile([128, NCK, HPC, 2, D2], BF16, tag="wkv")
            nc.sync.dma_start(
                out=wk_sb, in_=wk.rearrange("(ck p) h pm m -> p ck h pm m", p=128)
            )
            for h in range(HPC):
                pk = [ps_proj.tile([128, 512], F32, tag=f"pq{sb}", name=f"pk{sb}") for sb in range(4)]
                n = 2 * NCK
                i = 0
                for ck in range(NCK):
                    for pm in range(2):
                        for sb in range(4):
                            nc.tensor.matmul(
                                pk[sb],
                                wk_sb[:, ck, h, pm, :],
                                ct[pm][:, ck, sb * 512:(sb + 1) * 512],
                                start=(i == 0),
                                stop=(i == n - 1),
                            )
                        i += 1
                for sb in range(4):
                    nc.vector.tensor_copy(out=ks[:, h, sb * 512:(sb + 1) * 512], in_=pk[sb])
            wv_sb = wkv_pool.tile([128, NCK, 2, HPC * D2], BF16, tag="wkv", name="wv_sb")
            nc.sync.dma_start(
                out=wv_sb, in_=wv.rearrange("(ck p) pm n -> p ck pm n", p=128)
            )
            for st in range(NST):
                pv = ps_v.tile([128, 512], F32, tag="pv")
                n = 2 * NCK
                i = 0
                for ck in range(NCK):
                    for pm in range(2):
                        nc.tensor.matmul(
                            pv,
                            ct[pm][:, ck, st * 128:(st + 1) * 128],
                            wv_sb[:, ck, pm, :],
                            start=(i == 0),
                            stop=(i == n - 1),
                        )
                        i += 1
                nc.any.tensor_copy(out=vs[:, st, :], in_=pv)

        # ---------- P4: attention ----------
        if max_phase < 4:
            return
        ones = singles.tile([128, D2], EXP_DT)
        nc.vector.memset(ones, 1.0)
        ot = attn_sb.tile([128, HPC, L], OT_DT)  # [d2, h, l]

        with (
            tc.tile_pool(name="exp", bufs=2) as exp_pool,
            tc.tile_pool(name="esum", bufs=2) as esum_pool,
            tc.tile_pool(name="ps_s", bufs=2, space="PSUM") as ps_s,
            tc.tile_pool(name="ps_d", bufs=2, space="PSUM") as ps_d,
            tc.tile_pool(name="ps_o", bufs=2, space="PSUM") as ps_o,
        ):
            for h in range(HPC):
                for lb in range(NLB):
                    lsl = slice(lb * 512, (lb + 1) * 512)
                    expt = exp_pool.tile([128, NST, 512], EXP_DT, tag="expt")
                    for st in range(NST):
                        pscore = ps_s.tile([128, 512], F32, tag="pscore")
                        nc.tensor.matmul(
                            pscore,
                            ks[:, h, st * 128:(st + 1) * 128],
                            qs[:, h, lsl],
                            start=True,
                            stop=True,
                        )
                        nc.scalar.activation(
                            out=expt[:, st, :],
                            in_=pscore,
                            func=mybir.ActivationFunctionType.Exp,
                            scale=SCALE,
                        )
                    # pairwise tree-sum of the 16 s-tiles
                    tree = esum_pool.tile([128, 8, 512], EXP_DT, tag="tree")
                    for j in range(8):
                        nc.any.tensor_add(
                            out=tree[:, j, :], in0=expt[:, 2 * j, :], in1=expt[:, 2 * j + 1, :]
                        )
                    for span in (4, 2, 1):
                        for j in range(span):
                            nc.any.tensor_add(
                                out=tree[:, j, :], in0=tree[:, j, :], in1=tree[:, j + span, :]
                            )
                    pden = ps_d.tile([128, 512], F32, tag="pden")
                    nc.tensor.matmul(pden, ones, tree[:, 0, :], start=True, stop=True)
                    recip = esum_pool.tile([128, 512], F32, tag="recip")
                    nc.vector.reciprocal(out=recip, in_=pden)
                    # av: OT[d2, l] accumulated over s-tiles
                    pav = ps_o.tile([128, 512], F32, tag="pav")
                    for st in range(NST):
                        nc.tensor.matmul(
                            pav,
                            vs[:, st, h * D2:(h + 1) * D2],
                            expt[:, st, :],
                            start=(st == 0),
                            stop=(st == NST - 1),
                        )
                    nc.vector.tensor_mul(out=ot[:, h, lsl], in0=pav, in1=recip)

        # ---------- P5: output projection ----------
        if max_phase < 5:
            return
        with (
            tc.tile_pool(name="wo", bufs=1) as wo_pool,
            tc.tile_pool(name="ysb", bufs=4) as ysb_pool,
            tc.tile_pool(name="ps_y", bufs=2, space="PSUM") as ps_y,
        ):
            wo_sb = wo_pool.tile([128, HPC, 2, NEB, 512], OT_DT, tag="wo")
            with tc.tile_pool(name="wo_stage", bufs=2) as wo_stage:
                for h in range(HPC):
                    st32 = wo_stage.tile([128, 2, NEB, 512], F32, tag="st32")
                    nc.sync.dma_start(out=st32, in_=wo[h].rearrange("p ri eb e -> p ri eb e"))
                    nc.vector.tensor_copy(out=wo_sb[:, h], in_=st32)
            for lt in range(NLT):
                py = [
                    ps_y.tile([128, 512], F32, tag=f"py{ri}{eb}", name=f"py{ri}{eb}")
                    for ri in range(2)
                    for eb in range(NEB)
                ]
                for h in range(HPC):
                    k = 0
                    for ri in range(2):
                        for eb in range(NEB):
                            nc.tensor.matmul(
                                py[k],
                                ot[:, h, lt * 128:(lt + 1) * 128],
                                wo_sb[:, h, ri, eb, :],
                                start=(h == 0),
                                stop=(h == HPC - 1),
                            )
                            k += 1
                lrow = slice(lt * 128, (lt + 1) * 128)
                for eb in range(NEB):
                    esl = slice(eb * 512, (eb + 1) * 512)
                    yr_t = ysb_pool.tile([128, 512], F32, tag="yrt")
                    nc.any.tensor_copy(out=yr_t, in_=py[eb])
                    nc.sync.dma_start(out=y_r[lrow, esl], in_=yr_t)
                    yi_t = ysb_pool.tile([128, 512], F32, tag="yit")
                    nc.any.tensor_copy(out=yi_t, in_=py[NEB + eb])
                    nc.sync.dma_start(out=y_i[lrow, esl], in_=yi_t)


def _prep_core_inputs(inputs, core):
    """Slice + host-prepare the weight layouts for one core."""
    import ml_dtypes

    b = core // 4
    g = core % 4
    hcols = slice(g * HPC * D, (g + 1) * HPC * D)  # 256 channel cols/rows

    wq_r = inputs["wq_r"][:, hcols]
    wq_i = inputs["wq_i"][:, hcols]
    wk_r = inputs["wk_r"][:, hcols]
    wk_i = inputs["wk_i"][:, hcols]
    wv_r = inputs["wv_r"][:, hcols]
    wv_i = inputs["wv_i"][:, hcols]
    wo_r = inputs["wo_r"][hcols, :]
    wo_i = inputs["wo_i"][hcols, :]

    def stack_lhst(wr, wi):
        # [C, HPC, 2, D2]: pm=0 -> [wr | wi], pm=1 -> [-wi | wr]
        out = np.empty((C, HPC, 2, D2), np.float32)
        for hh in range(HPC):
            cs = slice(hh * D, (hh + 1) * D)
            out[:, hh, 0, :D] = wr[:, cs]
            out[:, hh, 0, D:] = wi[:, cs]
            out[:, hh, 1, :D] = -wi[:, cs]
            out[:, hh, 1, D:] = wr[:, cs]
        return out.astype(ml_dtypes.bfloat16)

    def stack_rhs_v(wr, wi):
        # [C, 2, HPC*D2]
        out = np.empty((C, 2, HPC * D2), np.float32)
        for hh in range(HPC):
            cs = slice(hh * D, (hh + 1) * D)
            out[:, 0, hh * D2:hh * D2 + D] = wr[:, cs]
            out[:, 0, hh * D2 + D:(hh + 1) * D2] = wi[:, cs]
            out[:, 1, hh * D2:hh * D2 + D] = -wi[:, cs]
            out[:, 1, hh * D2 + D:(hh + 1) * D2] = wr[:, cs]
        return out.astype(ml_dtypes.bfloat16)

    def stack_wo(wr, wi):
        # [HPC, D2, 2, NEB, 512]; rows 0:D multiply Or, D:D2 multiply Oi
        out = np.empty((HPC, D2, 2, NEB, 512), np.float32)
        for hh in range(HPC):
            rs = slice(hh * D, (hh + 1) * D)
            for eb in range(NEB):
                esl = slice(eb * 512, (eb + 1) * 512)
                out[hh, :D, 0, eb, :] = wr[rs, esl]
                out[hh, D:, 0, eb, :] = -wi[rs, esl]
                out[hh, :D, 1, eb, :] = wi[rs, esl]
                out[hh, D:, 1, eb, :] = wr[rs, esl]
        return out

    return {
        "x_r": np.ascontiguousarray(inputs["inputs_real"][b]),
        "x_i": np.ascontiguousarray(inputs["inputs_imag"][b]),
        "c_r": np.ascontiguousarray(inputs["context_real"][b]),
        "c_i": np.ascontiguousarray(inputs["context_imag"][b]),
        "wq": stack_lhst(wq_r, wq_i),
        "wk": stack_lhst(wk_r, wk_i),
        "wv": stack_rhs_v(wv_r, wv_i),
        "wo": stack_wo(wo_r, wo_i),
    }


def get_program():
    if "nc" not in _CACHE:
        _CACHE["nc"] = _build_program()
    return _CACHE["nc"]


def kernel(**inputs):
    nc = get_program()
    in_maps = [_prep_core_inputs(inputs, core) for core in range(8)]
    res = run_bass_kernel_spmd(nc, in_maps, core_ids=list(range(8)))

    yr = np.zeros((B, L, C), np.float32)
    yi = np.zeros((B, L, C), np.float32)
    for core in range(8):
        b = core // 4
        yr[b] += res.results[core]["y_r"]
        yi[b] += res.results[core]["y_i"]
    yr += inputs["bo_r"][None, None, :]
    yi += inputs["bo_i"][None, None, :]
    return np.stack([yr, yi], axis=0)


# revision 16
# speedup vs baseline: 28541.2684x; 28541.2684x over previous
"""Trainium2 Bass kernel for nn_ComplexCrossAttention.

Sharding: 8 cores = 2 batches x 4 head-groups (4 heads each).
Each core computes, for its (b, head-group):
  - complex Q/K/V projections (column-sharded by head) in transposed layout
  - attention scoresT = (qr.kr + qi.ki)*scale with s on partitions
  - softmax (no max-subtraction; scores are provably small) via exp + column-sum
  - av in transposed layout -> OT [d2, l]
  - partial output projection (row-sharded by head)
Host sums the 4 partial y per batch and adds the bias.

All matmuls are N=512 full-rate. Activations enter as bf16 (DMA-xbar
transpose requires 2-byte dtype) via gpsimd cast-DMAs; the scores path
uses float32r (TF32-like, full rate at N>=256).
"""

import sys

import numpy as np

try:
    import concourse.bacc as bacc
except ImportError:  # pragma: no cover - fallback for bare environments
    sys.path.insert(0, "/opt/trn_rl_repo")
    import concourse.bacc as bacc

import concourse.mybir as mybir
import concourse.tile as tile
from concourse.bass_utils import run_bass_kernel_spmd

F32 = mybir.dt.float32
BF16 = mybir.dt.bfloat16
F32R = mybir.dt.float32r

# ---- problem constants (hardcoded per contract) ----
B, L, S, C = 2, 2048, 2048, 1024
H, D = 16, 64
SCALE = float(1.0 / np.sqrt(np.float32(D)))
HPC = 4          # heads per core
D2 = 2 * D       # stacked (real|imag) head dim = 128
NCK = C // 128   # contraction chunks = 8
NLB = L // 512   # l-blocks = 4
NST = S // 128   # s-tiles = 16
NLT = L // 128   # l-tiles = 16
NEB = 2          # e-blocks of 512 in C

# ---- dtype configuration ----
QS_DT = F32R     # Qs/Ks (scores operands)
EXP_DT = BF16    # expT / Vs / ones (av + denom operands)
VS_DT = EXP_DT
OT_DT = BF16     # OT / wo (o-proj operands)

_CACHE = {}


def _build_program():
    nc = bacc.Bacc("TRN2", target_bir_lowering=False, debug=False, num_devices=8)

    # per-core external inputs
    x_r = nc.dram_tensor("x_r", [L, C], F32, kind="ExternalInput")
    x_i = nc.dram_tensor("x_i", [L, C], F32, kind="ExternalInput")
    c_r = nc.dram_tensor("c_r", [S, C], F32, kind="ExternalInput")
    c_i = nc.dram_tensor("c_i", [S, C], F32, kind="ExternalInput")
    # stacked complex projection weights (host-prepared, bf16)
    # wq/wk: [C, HPC, 2, 128]  (c, head, pm, m) ; lhsT tiles
    wq = nc.dram_tensor("wq", [C, HPC, 2, D2], BF16, kind="ExternalInput")
    wk = nc.dram_tensor("wk", [C, HPC, 2, D2], BF16, kind="ExternalInput")
    # wv: [C, 2, HPC*128]  (c, pm, all-head d2) ; rhs tiles
    wv = nc.dram_tensor("wv", [C, 2, HPC * D2], BF16, kind="ExternalInput")
    # wo: [HPC, 128, 2, NEB, 512]  (head, d2row, ri, eblock, e) ; rhs tiles
    wo = nc.dram_tensor("wo", [HPC, D2, 2, NEB, 512], OT_DT, kind="ExternalInput")

    y_r = nc.dram_tensor("y_r", [L, C], F32, kind="ExternalOutput")
    y_i = nc.dram_tensor("y_i", [L, C], F32, kind="ExternalOutput")

    with tile.TileContext(nc) as tc:
        _emit(nc, tc, x_r, x_i, c_r, c_i, wq, wk, wv, wo, y_r, y_i)

    nc.compile()
    return nc


def _emit(nc, tc, x_r, x_i, c_r, c_i, wq, wk, wv, wo, y_r, y_i):
    from contextlib import ExitStack

    ctx = ExitStack()
    with ctx:
        singles = ctx.enter_context(tc.tile_pool(name="singles", bufs=1))
        attn_sb = ctx.enter_context(tc.tile_pool(name="attn_sb", bufs=1))
        dram = ctx.enter_context(tc.tile_pool(name="dram", bufs=1, space="DRAM"))

        # bf16 mirrors of the activations (cast-DMA'd, then xbar-transposed)
        xbf = [dram.tile([L, C], BF16, tag=f"xbf{t}", name=f"xbf{t}") for t in range(2)]
        cbf = [dram.tile([S, C], BF16, tag=f"cbf{t}", name=f"cbf{t}") for t in range(2)]

        # persistent attention operands
        qs = attn_sb.tile([128, HPC, L], QS_DT)            # [d2, h, l]
        ks = attn_sb.tile([128, HPC, S], QS_DT)            # [d2, h, s]
        vs = attn_sb.tile([128, NST, HPC * D2], VS_DT)     # [s-part, st, d2all]

        # ---------- P0a: cast x fp32 -> bf16 (DRAM->DRAM compute DMA) ----------
        # column-halves so transposes of early chunks can start sooner
        for asrc, adst in ((x_r, xbf[0]), (x_i, xbf[1]),
                           (c_r, cbf[0]), (c_i, cbf[1])):
            for cb in range(4):
                csl = slice(cb * 256, (cb + 1) * 256)
                nc.gpsimd.dma_start(out=adst[:, csl], in_=asrc[:, csl])

        # ---------- P1+P2: transpose-in x, Q projection ----------
        with (
            tc.tile_pool(name="xt", bufs=1) as xt_pool,
            tc.tile_pool(name="wqk", bufs=1) as wqk_pool,
            tc.tile_pool(name="ps_proj", bufs=1, space="PSUM") as ps_proj,
        ):
            wq_sb = wqk_pool.tile([128, NCK, HPC, 2, D2], BF16, tag="wq")
            nc.sync.dma_start(
                out=wq_sb, in_=wq.rearrange("(ck p) h pm m -> p ck h pm m", p=128)
            )
            xt = [xt_pool.tile([128, NCK, L], BF16, tag=f"xt{t}", name=f"xt{t}") for t in range(2)]
            for ck in range(NCK):
                for t in range(2):
                    nc.sync.dma_start(
                        out=xt[t][:, ck, :],
                        in_=xbf[t][:, ck * 128:(ck + 1) * 128],
                        transpose=True,
                    )
            for hp in range(HPC // 2):
                pq = [
                    [ps_proj.tile([128, 512], F32, tag=f"pq{hh}{lb}", name=f"pq{hh}{lb}")
                     for lb in range(NLB)]
                    for hh in range(2)
                ]
                n = 2 * NCK
                i = 0
                for ck in range(NCK):
                    for pm in range(2):
                        for hh in range(2):
                            for lb in range(NLB):
                                nc.tensor.matmul(
                                    pq[hh][lb],
                                    wq_sb[:, ck, 2 * hp + hh, pm, :],
                                    xt[pm][:, ck, lb * 512:(lb + 1) * 512],
                                    start=(i == 0),
                                    stop=(i == n - 1),
                                )
                        i += 1
                for hh in range(2):
                    for lb in range(NLB):
                        nc.vector.tensor_copy(
                            out=qs[:, 2 * hp + hh, lb * 512:(lb + 1) * 512], in_=pq[hh][lb]
                        )

        # ---------- P3: transpose ctx, K and V projections ----------
        # score/exp pools open across P3 so the scheduler can hoist
        # scoresT+exp of early heads into K/V-phase gaps.
        exp_pool = ctx.enter_context(tc.tile_pool(name="exp", bufs=2))
        ps_s = ctx.enter_context(tc.tile_pool(name="ps_s", bufs=2, space="PSUM"))

        with (
            tc.tile_pool(name="ct", bufs=1) as ct_pool,
            tc.tile_pool(name="wkv", bufs=1) as wkv_pool,
            tc.tile_pool(name="ps_proj2", bufs=1, space="PSUM") as ps_proj,
            tc.tile_pool(name="ps_v", bufs=2, space="PSUM") as ps_v,
        ):
            wk_sb = wkv_pool.tile([128, NCK, HPC, 2, D2], BF16, tag="wkv")
            nc.sync.dma_start(
                out=wk_sb, in_=wk.rearrange("(ck p) h pm m -> p ck h pm m", p=128)
            )
            ct = [ct_pool.tile([128, NCK, S], BF16, tag=f"ct{t}", name=f"ct{t}") for t in range(2)]
            for ck in range(NCK):
                for t in range(2):
                    nc.sync.dma_start(
                        out=ct[t][:, ck, :],
                        in_=cbf[t][:, ck * 128:(ck + 1) * 128],
                        transpose=True,
                    )
            for h in range(HPC):
                pk = [ps_proj.tile([128, 512], F32, tag=f"pq{sb}", name=f"pk{sb}") for sb in range(4)]
                n = 2 * NCK
                i = 0
                for ck in range(NCK):
                    for pm in range(2):
                        for sb in range(4):
                            nc.tensor.matmul(
                                pk[sb],
                                wk_sb[:, ck, h, pm, :],
                                ct[pm][:, ck, sb * 512:(sb + 1) * 512],
                                start=(i == 0),
                                stop=(i == n - 1),
                            )
                        i += 1
                for sb in range(4):
                    nc.vector.tensor_copy(out=ks[:, h, sb * 512:(sb + 1) * 512], in_=pk[sb])
            wv_sb = wkv_pool.tile([128, NCK, 2, HPC * D2], BF16, tag="wkv", name="wv_sb")
            nc.sync.dma_start(
                out=wv_sb, in_=wv.rearrange("(ck p) pm n -> p ck pm n", p=128)
            )
            for st in range(NST):
                pv = ps_v.tile([128, 512], F32, tag="pv")
                n = 2 * NCK
                i = 0
                for ck in range(NCK):
                    for pm in range(2):
                        nc.tensor.matmul(
                            pv,
                            ct[pm][:, ck, st * 128:(st + 1) * 128],
                            wv_sb[:, ck, pm, :],
                            start=(i == 0),
                            stop=(i == n - 1),
                        )
                        i += 1
                nc.vector.tensor_copy(out=vs[:, st, :], in_=pv)

        # ---------- P4+P5 fused: attention + output projection, lb-outer ----------
        with (
            tc.tile_pool(name="late", bufs=1) as late_pool,
            tc.tile_pool(name="exp2", bufs=3) as exp_pool2,
            tc.tile_pool(name="otp", bufs=2) as ot_pool,
            tc.tile_pool(name="ysb", bufs=4) as ysb_pool,
            tc.tile_pool(name="ps_d", bufs=1, space="PSUM") as ps_d,
            tc.tile_pool(name="ps_o", bufs=2, space="PSUM") as ps_o,
            tc.tile_pool(name="ps_y", bufs=1, space="PSUM") as ps_y,
        ):
            ones = late_pool.tile([128, D2], EXP_DT)
            nc.vector.memset(ones, 1.0)
            wo_sb = late_pool.tile([128, HPC, 2, NEB, 512], OT_DT, tag="wo", name="wo_sb")
            nc.sync.dma_start(out=wo_sb, in_=wo.rearrange("h p ri eb e -> p h ri eb e"))
            for lb in range(NLB):
                lsl = slice(lb * 512, (lb + 1) * 512)
                ot = ot_pool.tile([128, HPC, 512], OT_DT, tag="ot", name="ot")
                for h in range(HPC):
                    pool_h = exp_pool if lb == 0 else exp_pool2
                    expt = pool_h.tile([128, NST, 512], EXP_DT, tag="expt", name="expt")
                    for st in range(NST):
                        pscore = ps_s.tile([128, 512], F32, tag="pscore")
                        nc.tensor.matmul(
                            pscore,
                            ks[:, h, st * 128:(st + 1) * 128],
                            qs[:, h, lsl],
                            start=True,
                            stop=True,
                        )
                        nc.scalar.activation(
                            out=expt[:, st, :],
                            in_=pscore,
                            func=mybir.ActivationFunctionType.Exp,
                            scale=SCALE,
                        )
                    # av: OT[d2, l] accumulated over s-tiles (reads expt first)
                    pav = ps_o.tile([128, 512], F32, tag="pav")
                    for st in range(NST):
                        nc.tensor.matmul(
                            pav,
                            vs[:, st, h * D2:(h + 1) * D2],
                            expt[:, st, :],
                            start=(st == 0),
                            stop=(st == NST - 1),
                        )
                    # in-place pairwise tree-sum of the 16 s-tiles (WAR after av)
                    for step in (1, 2, 4, 8):
                        eng = nc.gpsimd if step == 1 else nc.vector
                        for j in range(0, NST, 2 * step):
                            eng.tensor_add(
                                out=expt[:, j, :], in0=expt[:, j, :], in1=expt[:, j + step, :]
                            )
                    pden = ps_d.tile([128, 512], F32, tag="pden")
                    nc.tensor.matmul(pden, ones, expt[:, 0, :], start=True, stop=True)
                    recip = ot_pool.tile([128, 512], F32, tag="recip")
                    nc.vector.reciprocal(out=recip, in_=pden)
                    nc.vector.tensor_mul(out=ot[:, h, :], in0=pav, in1=recip)

                # output projection for this l-block (needs all heads' ot)
                for jt in range(4):
                    lt = lb * 4 + jt
                    lrow = slice(lt * 128, (lt + 1) * 128)
                    for eb in range(NEB):
                        esl = slice(eb * 512, (eb + 1) * 512)
                        py = [ps_y.tile([128, 512], F32, tag=f"py{ri}", name=f"py{ri}")
                              for ri in range(2)]
                        for h in range(HPC):
                            for ri in range(2):
                                nc.tensor.matmul(
                                    py[ri],
                                    ot[:, h, jt * 128:(jt + 1) * 128],
                                    wo_sb[:, h, ri, eb, :],
                                    start=(h == 0),
                                    stop=(h == HPC - 1),
                                )
                        yr_t = ysb_pool.tile([128, 512], F32, tag="yrt")
                        nc.vector.tensor_copy(out=yr_t, in_=py[0])
                        nc.sync.dma_start(out=y_r[lrow, esl], in_=yr_t)
                        yi_t = ysb_pool.tile([128, 512], F32, tag="yit")
                        nc.vector.tensor_copy(out=yi_t, in_=py[1])
                        nc.sync.dma_start(out=y_i[lrow, esl], in_=yi_t)


def _prep_core_inputs(inputs, core):
    """Slice + host-prepare the weight layouts for one core."""
    import ml_dtypes

    b = core // 4
    g = core % 4
    hcols = slice(g * HPC * D, (g + 1) * HPC * D)  # 256 channel cols/rows

    wq_r = inputs["wq_r"][:, hcols]
    wq_i = inputs["wq_i"][:, hcols]
    wk_r = inputs["wk_r"][:, hcols]
    wk_i = inputs["wk_i"][:, hcols]
    wv_r = inputs["wv_r"][:, hcols]
    wv_i = inputs["wv_i"][:, hcols]
    wo_r = inputs["wo_r"][hcols, :]
    wo_i = inputs["wo_i"][hcols, :]

    def stack_lhst(wr, wi):
        # [C, HPC, 2, D2]: pm=0 -> [wr | wi], pm=1 -> [-wi | wr]
        out = np.empty((C, HPC, 2, D2), np.float32)
        for hh in range(HPC):
            cs = slice(hh * D, (hh + 1) * D)
            out[:, hh, 0, :D] = wr[:, cs]
            out[:, hh, 0, D:] = wi[:, cs]
            out[:, hh, 1, :D] = -wi[:, cs]
            out[:, hh, 1, D:] = wr[:, cs]
        return out.astype(ml_dtypes.bfloat16)

    def stack_rhs_v(wr, wi):
        # [C, 2, HPC*D2]
        out = np.empty((C, 2, HPC * D2), np.float32)
        for hh in range(HPC):
            cs = slice(hh * D, (hh + 1) * D)
            out[:, 0, hh * D2:hh * D2 + D] = wr[:, cs]
            out[:, 0, hh * D2 + D:(hh + 1) * D2] = wi[:, cs]
            out[:, 1, hh * D2:hh * D2 + D] = -wi[:, cs]
            out[:, 1, hh * D2 + D:(hh + 1) * D2] = wr[:, cs]
        return out.astype(ml_dtypes.bfloat16)

    def stack_wo(wr, wi):
        # [HPC, D2, 2, NEB, 512]; rows 0:D multiply Or, D:D2 multiply Oi
        out = np.empty((HPC, D2, 2, NEB, 512), np.float32)
        for hh in range(HPC):
            rs = slice(hh * D, (hh + 1) * D)
            for eb in range(NEB):
                esl = slice(eb * 512, (eb + 1) * 512)
                out[hh, :D, 0, eb, :] = wr[rs, esl]
                out[hh, D:, 0, eb, :] = -wi[rs, esl]
                out[hh, :D, 1, eb, :] = wi[rs, esl]
                out[hh, D:, 1, eb, :] = wr[rs, esl]
        return out.astype(ml_dtypes.bfloat16)

    return {
        "x_r": np.ascontiguousarray(inputs["inputs_real"][b]),
        "x_i": np.ascontiguousarray(inputs["inputs_imag"][b]),
        "c_r": np.ascontiguousarray(inputs["context_real"][b]),
        "c_i": np.ascontiguousarray(inputs["context_imag"][b]),
        "wq": stack_lhst(wq_r, wq_i),
        "wk": stack_lhst(wk_r, wk_i),
        "wv": stack_rhs_v(wv_r, wv_i),
        "wo": stack_wo(wo_r, wo_i),
    }


def get_program():
    if "nc" not in _CACHE:
        _CACHE["nc"] = _build_program()
    return _CACHE["nc"]


def kernel(**inputs):
    nc = get_program()
    in_maps = [_prep_core_inputs(inputs, core) for core in range(8)]
    res = run_bass_kernel_spmd(nc, in_maps, core_ids=list(range(8)))

    yr = np.zeros((B, L, C), np.float32)
    yi = np.zeros((B, L, C), np.float32)
    for core in range(8):
        b = core // 4
        yr[b] += res.results[core]["y_r"]
        yi[b] += res.results[core]["y_i"]
    yr += inputs["bo_r"][None, None, :]
    yi += inputs["bo_i"][None, None, :]
    return np.stack([yr, yi], axis=0)
